# revision 6
# baseline (speedup 1.0000x reference)
"""Trainium2 Bass kernel for nn_CLTBernoulliDecoder (CLT Bernoulli decoder loss).

Reference computation:
    logits = (z @ W + b).reshape(Bz, F, 2)        # interleaved states
    root fix: logits[:, root, 0] := logits[:, root, 1]
    xt = x[:, tree] ;  x_cond = stack([1-xt, xt])
    ls, lsn = log_sigmoid(+-logits)
    out[b,i] = sum_{j,s} x_cond*x * ls + x_cond*(1-x) * lsn

Algebraic restructuring used here (exact, not an approximation):
    log_sigmoid(t) = t - softplus(t)
    =>  out[b,i] = G[b,:]@z[i,:] + h[b]                (linear term, folded through W)
                 - sum_j SP0[i,j]                      (softplus of state-0 logits)
                 + sum_j xt'[b,j] * (SP0 - SP1)[i,j]   (parent-weighted difference)
    where SP_s = softplus(z @ W_s + b_s)  (W_s = W[:, s::2]),
          xt'[b,j] = 1 at roots else x[b, tree[j]],
          G = A_hat @ W.T,  h = A_hat @ b,
          A_hat[b, 2j+s] interleaves ((1-xt')*x, xt'*x).
    The root fix is exactly equivalent to setting xt' = 1 at root features.

softplus is evaluated as Ln(1 + Exp(l)) -- exp and ln share one ACT table set.
Biases ride along the matmuls as a 65th contraction row (z' has a ones row).

Sharding: data-parallel over Bz (4096 -> 8 x 512). x-derived coefficient
matrices are replicated; per-core outputs [256, 512] are concatenated on
axis 1 to form the full [256, 4096] result.
"""

import numpy as np
import ml_dtypes

BF16 = ml_dtypes.bfloat16

# Problem dimensions (hardcoded per spec).
BX = 256          # data points
BZ = 4096         # latent samples
ZD = 64           # latent dim
F = 784           # features
FP = 896          # features padded to 7*128
NT = FP // 128    # 7 j-tiles
N_CORES = 8
BZS = BZ // N_CORES  # 512 per core

# j-tile chunks processed per ACT op (PSUM budget: 2 banks per state)
CHUNKS = [(0, 2), (2, 4), (4, 6), (6, 7)]

_CACHE = {}


def _build_bass():
    import concourse.bass as bass
    import concourse.mybir as mybir
    import concourse.tile as tile
    from concourse import bacc
    from concourse.hw_specs import get_activation_tables

    fp32 = mybir.dt.float32
    bf16 = mybir.dt.bfloat16
    EXP = mybir.ActivationFunctionType.Exp
    LN = mybir.ActivationFunctionType.Ln

    class _Bacc(bacc.Bacc):
        """Pin Exp and Ln to the one table set holding both, so the table
        is loaded once instead of ping-ponging between per-function sets
        (~1.3us per reload)."""

        def insert_act_table_loads(self):
            import concourse._compat as _compat
            from concourse import hw_specs as _hs

            has_activation = any(
                isinstance(i, mybir.InstActivation)
                for b in self.main_func.blocks
                for i in b.instructions
            )
            if not has_activation:
                return
            tables = []
            for name, funcs in get_activation_tables(self.m.arch).items():
                if name != "natural_log_exp_and_others":
                    funcs = {f for f in funcs
                             if f not in (EXP, LN)}
                tables.append((name, funcs))
            import bass_rust as _bass_rust
            _bass_rust.insert_act_table_loads(self, tables)

    nc = _Bacc(None, target_bir_lowering=False)

    d_w01 = nc.dram_tensor("w01", [ZD + 1, 2, FP], bf16, kind="ExternalInput")
    d_zp = nc.dram_tensor("zp", [ZD + 1, BZS], bf16, kind="ExternalInput")
    d_gp = nc.dram_tensor("gp", [ZD + 1, BX], bf16, kind="ExternalInput")
    d_xtt = nc.dram_tensor("xtt", [128, NT, BX], bf16, kind="ExternalInput")
    d_msk = nc.dram_tensor("msk", [128, NT, 1], bf16, kind="ExternalInput")
    d_neg = nc.dram_tensor("neg", [1, BX], bf16, kind="ExternalInput")
    d_out = nc.dram_tensor("out", [BX, BZS], fp32, kind="ExternalOutput")

    with tile.TileContext(nc) as tc:
        with (
            tc.tile_pool(name="singles", bufs=1) as singles,
            tc.tile_pool(name="outs", bufs=2) as outs_pool,
            tc.tile_pool(name="psum_l", bufs=1, space="PSUM") as psum_l,
            tc.tile_pool(name="psum_o", bufs=1, space="PSUM") as psum_o,
        ):
            # ---- load inputs into SBUF (order = consumption order) ----
            zp = singles.tile([ZD + 1, BZS], bf16)
            nc.sync.dma_start(out=zp, in_=d_zp[:])
            w01 = singles.tile([ZD + 1, 2, FP], bf16)
            nc.sync.dma_start(out=w01, in_=d_w01[:])
            gp = singles.tile([ZD + 1, BX], bf16)
            nc.sync.dma_start(out=gp, in_=d_gp[:])
            xtt = singles.tile([128, NT, BX], bf16)
            nc.sync.dma_start(out=xtt, in_=d_xtt[:])
            msk = singles.tile([128, NT, 1], bf16)
            nc.sync.dma_start(out=msk, in_=d_msk[:])
            neg = singles.tile([1, BX], bf16)
            nc.sync.dma_start(out=neg, in_=d_neg[:])

            # ---- persistent accumulators / staging ----
            out_ps = [psum_o.tile([128, BZS], fp32, tag=f"out{m}", name=f"out_ps{m}")
                      for m in range(2)]
            s0_ps = psum_o.tile([1, BZS], fp32)
            e_all = singles.tile([128, 2, NT * BZS], fp32)
            sp_all = singles.tile([128, 2, NT * BZS], bf16)
            dd_all = singles.tile([128, NT * BZS], bf16)

            # linear term first: out[m] = G'[:, m-cols].T @ z'   (K = 65)
            for m in range(2):
                nc.tensor.matmul(out_ps[m], gp[:, m * 128:(m + 1) * 128],
                                 zp, start=True, stop=False)

            def emit_exp_chunk(ta, tb):
                sl = slice(ta * BZS, tb * BZS)
                w = (tb - ta) * BZS
                # logits (bias folded in as contraction row 64): [128 j, w i]
                l0 = psum_l.tile([128, w], fp32, tag="l0", name="l0")
                l1 = psum_l.tile([128, w], fp32, tag="l1", name="l1")
                for k, t in enumerate(range(ta, tb)):
                    ks = slice(k * BZS, (k + 1) * BZS)
                    nc.tensor.matmul(l0[:, ks], w01[:, 0, t * 128:(t + 1) * 128],
                                     zp, start=True, stop=True)
                    nc.tensor.matmul(l1[:, ks], w01[:, 1, t * 128:(t + 1) * 128],
                                     zp, start=True, stop=True)
                # e = exp(l) (PSUM -> SBUF f32)
                nc.scalar.activation(e_all[:, 0, sl], l0, EXP)
                nc.scalar.activation(e_all[:, 1, sl], l1, EXP)

            def emit_ln_chunk(ta, tb):
                sl = slice(ta * BZS, tb * BZS)
                # sp = ln(e + 1) -> bf16, then D = SP0 - SP1
                nc.scalar.activation(sp_all[:, 0, sl], e_all[:, 0, sl], LN, bias=1.0)
                nc.scalar.activation(sp_all[:, 1, sl], e_all[:, 1, sl], LN, bias=1.0)
                nc.vector.tensor_sub(dd_all[:, sl], sp_all[:, 0, sl], sp_all[:, 1, sl])
                for t in range(ta, tb):
                    ts = slice(t * BZS, (t + 1) * BZS)
                    # masked column-sum of SP0
                    nc.tensor.matmul(s0_ps, msk[:, t, :], sp_all[:, 0, ts],
                                     start=(t == 0), stop=(t == NT - 1))
                    # main: out[b, i] += sum_j xt'[j, b] * D[j, i]
                    for m in range(2):
                        nc.tensor.matmul(out_ps[m], xtt[:, t, m * 128:(m + 1) * 128],
                                         dd_all[:, ts], start=False, stop=False)

            emit_exp_chunk(0, 2)
            emit_exp_chunk(2, 4)
            emit_ln_chunk(0, 4)
            emit_exp_chunk(4, 6)
            emit_exp_chunk(6, 7)
            emit_ln_chunk(4, 7)

            # spread -sum_j SP0 over all rows: out += (-1)[b] (x) s0[i]
            s0_sb = singles.tile([1, BZS], bf16)
            nc.vector.tensor_copy(s0_sb, s0_ps)
            for m in range(2):
                nc.tensor.matmul(out_ps[m], neg[:, m * 128:(m + 1) * 128],
                                 s0_sb, start=False, stop=True)

            # evict
            for m in range(2):
                o = outs_pool.tile([128, BZS], fp32, tag="o", name="o")
                nc.vector.tensor_copy(o, out_ps[m])
                nc.sync.dma_start(out=d_out[m * 128:(m + 1) * 128, :], in_=o)

    nc.compile()
    return nc


def _host_prep(x, z, W, b, tree):
    x = np.asarray(x, dtype=np.float32)
    z = np.asarray(z, dtype=np.float32)
    W = np.asarray(W, dtype=np.float32)
    b = np.asarray(b, dtype=np.float32)
    tree = np.asarray(tree, dtype=np.int64)

    root = tree < 0
    xt = x[:, tree]              # -1 wraps to last column, same as the ref
    xt[:, root] = 1.0            # root fix folded into coefficients

    # A_hat (interleaved): a0 = (1-xt')*x, a1 = xt'*x  (root rows give (0, x))
    Ahat = np.empty((BX, 2 * F), dtype=np.float32)
    Ahat[:, 0::2] = (1.0 - xt) * x
    Ahat[:, 1::2] = xt * x
    G = Ahat @ W.T               # [BX, ZD]
    h = Ahat @ b                 # [BX]

    # gp: [65, 256] = [G.T; h]
    gp = np.zeros((ZD + 1, BX), dtype=np.float32)
    gp[:ZD] = G.T
    gp[ZD] = h
    gp = gp.astype(BF16)

    # w01: [65, 2, 896] de-interleaved, bias as row 64, zero padded
    w01 = np.zeros((ZD + 1, 2, FP), dtype=np.float32)
    w01[:ZD, 0, :F] = W[:, 0::2]
    w01[:ZD, 1, :F] = W[:, 1::2]
    w01[ZD, 0, :F] = b[0::2]
    w01[ZD, 1, :F] = b[1::2]
    w01 = w01.astype(BF16)

    # xtt: [128, 7, 256]: [p, t, b] = xt'[b, t*128+p] (0 pad)
    xtt = np.zeros((FP, BX), dtype=np.float32)
    xtt[:F] = xt.T
    xtt = np.ascontiguousarray(
        xtt.reshape(NT, 128, BX).transpose(1, 0, 2)).astype(BF16)

    # msk: [128, 7, 1] 1 for real features
    msk = np.zeros((FP,), dtype=np.float32)
    msk[:F] = 1.0
    msk = np.ascontiguousarray(
        msk.reshape(NT, 128, 1).transpose(1, 0, 2)).astype(BF16)

    neg = np.full((1, BX), -1.0, dtype=np.float32).astype(BF16)

    # z': [65, 4096] with ones row (bias channel)
    zp = np.ones((ZD + 1, BZ), dtype=np.float32)
    zp[:ZD] = z.T
    zp = zp.astype(BF16)

    rep = {"w01": w01, "gp": gp, "xtt": xtt, "msk": msk, "neg": neg}
    in_maps = []
    for c in range(N_CORES):
        m = dict(rep)
        m["zp"] = np.ascontiguousarray(zp[:, c * BZS:(c + 1) * BZS])
        in_maps.append(m)
    return in_maps


def kernel(x, z, W, b, tree, **_unused):
    from concourse.bass_utils import run_bass_kernel_spmd

    if "nc" not in _CACHE:
        _CACHE["nc"] = _build_bass()
    nc = _CACHE["nc"]

    import os
    in_maps = _host_prep(x, z, W, b, tree)
    res = run_bass_kernel_spmd(nc, in_maps, core_ids=list(range(N_CORES)),
                               tmpdir=os.environ.get("BASS_TMPDIR"))
    _CACHE["last_result"] = res
    out = np.concatenate([res.results[c]["out"] for c in range(N_CORES)], axis=1)
    return out.astype(np.float32)
